# revision 1
# baseline (speedup 1.0000x reference)
"""Trainium2 Bass kernel for CodecLlamaCodecEmbedding (MoE-routed per-codebook MLP).

Strategy (expert-parallel): there are 8 codebooks and 8 NeuronCores. The host
sorts tokens by codebook (the MoE dispatch) and sends core k exactly the tokens
belonging to codebook k (padded to a 128-aligned capacity so the SPMD program
is static), already gathered from the embedding table and transposed to
feature-major [16, cap] layout, plus that codebook's projector weights.

Each core then runs the 2-layer projector entirely on-device:
  layer 1:  hT = gelu(W1.T @ eT + b1)   feature-major [2048, cap], fp32 exact
            erf GELU on ScalarE with the bias fused into the activation.
  layer 2:  out[tok, :] = hT.T @ W2 + b2, accumulated over 16 K-chunks in
            PSUM; kc is the outer loop so the 16 MB W2 load streams from HBM
            directly into the accumulation, and each hT chunk is loaded into
            the PE array once per 4 matmuls (2048 moving columns).
Matmul operands use dtype float32r: full-rate PE streaming (4x faster than
plain fp32) at ~1e-4 relative error (measured on HW, ~15x tighter than bf16).
b2 is added on VectorE during the PSUM->SBUF copy, then [128, 512] blocks are
DMAd to DRAM. The host scatters the 8 per-core outputs back to token order.
"""

import math
from contextlib import ExitStack

import numpy as np

import concourse.bacc as bacc
import concourse.tile as tile
from concourse import mybir
from concourse.bass_utils import run_bass_kernel_spmd

# Problem constants (hardcoded per the harness contract).
NUM_CODEBOOKS = 8
CODEBOOK_SIZE = 2048
D = 16        # codebook embedding dim
H = 2048      # hidden size
V = NUM_CODEBOOKS * CODEBOOK_SIZE  # embed table rows
N_CORES = 8

P = 128                  # SBUF partitions / tile edge
CAP = 2304               # default token capacity per core (mean 2048, sigma ~42)
KC = H // P              # 16 contraction chunks for layer 2
NFREE = 512              # matmul moving-operand free dim (1 PSUM bank of fp32)
NSPLIT = H // NFREE      # 4 output column chunks

F32 = mybir.dt.float32
# float32r streams through the PE at full rate (1 cycle/row vs 4 for plain
# fp32 when the moving dim >= 256) with ~1e-4 relative error (measured on HW;
# ~15x more accurate than bf16). Same 4-byte IEEE storage, numpy side is f32.
F32R = mybir.dt.float32r

TUNE = {
    "group": 4,     # token tiles per layer-1 batch (N = group*128 matmuls)
    "ht_bufs": 5,
    "ob_bufs": 4,
    "l1_bufs": 2,
    "l2_bufs": 6,
    "w2_split": 4,  # W2 chunk DMA granularity (finer = smoother streaming)
}


def _emit(ctx: ExitStack, tc: tile.TileContext, aps: dict, nt: int,
          act=mybir.ActivationFunctionType.Gelu, tune=None, mm_dt=F32R, mm_dt2=None):
    mm_dt2 = mm_dt if mm_dt2 is None else mm_dt2
    t = dict(TUNE)
    t.update(tune or {})
    group = t["group"]
    nc = tc.nc
    et_ap = aps["et"]        # [D, cap] f32r, pre-gathered transposed embeddings
    w1_ap = aps["w1"]        # [D, H]  f32r
    b1_ap = aps["b1"]        # [P, KC] f32, b1_ap[p, c] = b1[c*128 + p]
    w2_ap = aps["w2"]        # [H, H]  f32r
    b2_ap = aps["b2"]        # [P, H]  f32, b2 replicated across partitions
    out_ap = aps["out"]      # [cap, H] f32

    const = ctx.enter_context(tc.tile_pool(name="const", bufs=1))
    w2p = ctx.enter_context(tc.tile_pool(name="w2p", bufs=1))
    htp = ctx.enter_context(tc.tile_pool(name="htp", bufs=t["ht_bufs"]))
    op = ctx.enter_context(tc.tile_pool(name="op", bufs=t["ob_bufs"]))
    l1p = ctx.enter_context(tc.tile_pool(name="l1p", bufs=t["l1_bufs"], space="PSUM"))
    l2p = ctx.enter_context(tc.tile_pool(name="l2p", bufs=t["l2_bufs"], space="PSUM"))

    # Small inputs first so they clear the DMA engines before the W2 stream.
    w1_sb = const.tile([D, H], mm_dt)
    nc.sync.dma_start(w1_sb[:], w1_ap[:, :])
    b1_sb = const.tile([P, KC], F32)
    nc.sync.dma_start(b1_sb[:], b1_ap[:, :])
    # The whole embedding block is tiny (16 x cap f32); land it before W2.
    et_sb = const.tile([D, nt * P], mm_dt)
    nc.sync.dma_start(et_sb[:], et_ap[:, :])
    # b2 (1 MB) is not needed until the first PSUM drains, ~40us in.
    b2_sb = const.tile([P, H], F32)
    nc.sync.dma_start(b2_sb[:], b2_ap[:, :])

    # W2 resident in SBUF: chunk kc holds rows [kc*128, (kc+1)*128) of W2,
    # laid out at columns [kc*H, (kc+1)*H). Streamed in chunk order; layer 2
    # consumes chunks in the same order, so compute starts before the load
    # finishes.
    w2_sb = w2p.tile([P, KC * H], mm_dt2)
    wsplit = t.get("w2_split", 1)
    for kc in range(KC):
        for s in range(wsplit):
            c0, c1 = s * (H // wsplit), (s + 1) * (H // wsplit)
            nc.sync.dma_start(
                w2_sb[:, kc * H + c0:kc * H + c1],
                w2_ap[kc * P:(kc + 1) * P, c0:c1],
            )

    # Balanced groups of <=`group` tiles, as equal as possible, so every
    # layer-1 matmul keeps a moving dim >= 256 (f32r full-rate region).
    n_groups = -(-nt // group)
    base, extra = divmod(nt, n_groups)
    sizes = [base + (1 if g < extra else 0) for g in range(n_groups)]
    starts = [sum(sizes[:g]) for g in range(n_groups)]
    for g0, gsz in zip(starts, sizes):
        tts = list(range(g0, g0 + gsz))
        w = len(tts) * P

        eT = et_sb[:, g0 * P:g0 * P + w]

        # Layer 1: hT[h, tok] = gelu(W1[:, h] . eT[:, tok] + b1[h]), stored
        # feature-major: ht tile [128 (h in chunk), 16 chunks x 128 tokens].
        hts = [htp.tile([P, H], mm_dt2, tag="ht", name=f"ht_{tt}") for tt in tts]
        for hc in range(KC):
            ps1 = l1p.tile([P, group * P], F32, tag="l1")
            nc.tensor.matmul(
                ps1[:, :w],
                w1_sb[:, hc * P:(hc + 1) * P],
                eT,
                start=True,
                stop=True,
            )
            for j in range(len(tts)):
                nc.scalar.activation(
                    hts[j][:, hc * P:(hc + 1) * P],
                    ps1[:, j * P:(j + 1) * P],
                    act,
                    bias=b1_sb[:, hc:hc + 1],
                )

        # Layer 2: out[tok, n] = sum_kc hT[kc][:, tok].T @ W2[kc][:, n] + b2[n]
        # kc outer: one hT weight load feeds 4 matmuls, and the first tiles
        # start as soon as the first W2 chunks land.
        pair = t.get("l2_pair", 1)
        for j0 in range(0, len(tts), pair):
            js = list(range(j0, min(j0 + pair, len(tts))))
            pss = {
                (j, n): l2p.tile([P, NFREE], F32, tag="l2", name=f"ps2_{tts[j]}_{n}")
                for j in js for n in range(NSPLIT)
            }
            # kc-major across the tile pair: the PE instruction stream consumes
            # W2 chunks in arrival order instead of tile 1 queueing behind
            # tile 0's last chunk.
            for kc in range(KC):
                for j in js:
                    for n in range(NSPLIT):
                        nc.tensor.matmul(
                            pss[j, n][:],
                            hts[j][:, kc * P:(kc + 1) * P],
                            w2_sb[:, kc * H + n * NFREE: kc * H + (n + 1) * NFREE],
                            start=(kc == 0),
                            stop=(kc == KC - 1),
                        )
            for j in js:
                tt = tts[j]
                for n in range(NSPLIT):
                    ob = op.tile([P, NFREE], F32, tag="ob")
                    nc.vector.tensor_add(ob[:], pss[j, n][:], b2_sb[:, n * NFREE:(n + 1) * NFREE])
                    nc.sync.dma_start(
                        out_ap[tt * P:(tt + 1) * P, n * NFREE:(n + 1) * NFREE], ob[:]
                    )


def build_nc(cap=CAP, act=mybir.ActivationFunctionType.Gelu, tune=None, mm_dt=F32R, mm_dt2=None):
    mm_dt2 = mm_dt if mm_dt2 is None else mm_dt2
    assert cap % P == 0 and cap > 0
    nt = cap // P
    nc = bacc.Bacc("TRN2", target_bir_lowering=False, debug=False)
    aps = {
        "et": nc.dram_tensor("et", [D, cap], mm_dt, kind="ExternalInput").ap(),
        "w1": nc.dram_tensor("w1", [D, H], mm_dt, kind="ExternalInput").ap(),
        "b1": nc.dram_tensor("b1", [P, KC], F32, kind="ExternalInput").ap(),
        "w2": nc.dram_tensor("w2", [H, H], mm_dt2, kind="ExternalInput").ap(),
        "b2": nc.dram_tensor("b2", [P, H], F32, kind="ExternalInput").ap(),
        "out": nc.dram_tensor("out", [cap, H], F32, kind="ExternalOutput").ap(),
    }
    with tile.TileContext(nc) as tc:
        with ExitStack() as ctx:
            _emit(ctx, tc, aps, nt, act=act, tune=tune, mm_dt=mm_dt, mm_dt2=mm_dt2)
    nc.compile()
    return nc


_NC_CACHE = {}


def _get_nc(cap=CAP):
    if cap not in _NC_CACHE:
        _NC_CACHE[cap] = build_nc(cap)
    return _NC_CACHE[cap]


def _gelu_exact_np(x):
    try:
        from scipy.special import erf
    except ImportError:
        erf = np.vectorize(math.erf)
    return 0.5 * x * (1.0 + erf(x / np.sqrt(2.0).astype(x.dtype)))


def _route(ids_flat: np.ndarray):
    """Sort token positions by codebook. Returns per-codebook position lists."""
    cb = ids_flat // CODEBOOK_SIZE
    order = np.argsort(cb, kind="stable")
    counts = np.bincount(cb, minlength=NUM_CODEBOOKS)
    starts = np.concatenate([[0], np.cumsum(counts)])
    return [order[starts[k]:starts[k + 1]] for k in range(NUM_CODEBOOKS)], counts


MAX_DEV_CAP = 4096  # beyond this (a ~48-sigma skew) overflow tokens go to host


def pick_cap(counts):
    """Smallest multiple of 128 covering the max per-codebook load."""
    need = max(int(counts.max()), P)
    nt = -(-need // P)
    return min(nt * P, MAX_DEV_CAP)


def make_in_maps(ids_flat, embed_table, W1, b1, W2, b2, cap=CAP):
    positions, counts = _route(ids_flat)
    table = np.ascontiguousarray(embed_table, dtype=np.float32)
    in_maps = []
    for k in range(NUM_CODEBOOKS):
        pos_k = positions[k][:cap]
        idx_pad = np.zeros(cap, np.int64)  # padding points at table row 0
        idx_pad[:len(pos_k)] = ids_flat[pos_k]
        in_maps.append({
            "et": np.ascontiguousarray(table[idx_pad].T),
            "w1": np.ascontiguousarray(W1[k], dtype=np.float32),
            "b1": np.ascontiguousarray(np.asarray(b1[k], dtype=np.float32).reshape(KC, P).T),
            "w2": np.ascontiguousarray(W2[k], dtype=np.float32),
            "b2": np.ascontiguousarray(
                np.broadcast_to(np.asarray(b2[k], dtype=np.float32), (P, H))
            ),
        })
    return in_maps, positions, counts


def kernel(codec_input_ids, embed_table, W1, b1, W2, b2):
    codec_input_ids = np.asarray(codec_input_ids)
    embed_table = np.asarray(embed_table, dtype=np.float32)
    W1 = np.asarray(W1, dtype=np.float32)
    b1 = np.asarray(b1, dtype=np.float32)
    W2 = np.asarray(W2, dtype=np.float32)
    b2 = np.asarray(b2, dtype=np.float32)

    B, S = codec_input_ids.shape
    ids_flat = codec_input_ids.reshape(-1).astype(np.int64)

    _, counts = _route(ids_flat)
    cap = pick_cap(counts)
    in_maps, positions, counts = make_in_maps(
        ids_flat, embed_table, W1, b1, W2, b2, cap=cap
    )

    try:
        nc = _get_nc(cap)
        results = run_bass_kernel_spmd(nc, in_maps, list(range(N_CORES))).results
    except Exception as e:  # device/compile fault: stay correct via host math
        import sys
        print(f"kernel: device path failed ({e!r}); host fallback", file=sys.stderr)
        results = None

    out_flat = np.zeros((B * S, H), np.float32)
    for k in range(NUM_CODEBOOKS):
        pos_k = positions[k]
        n_dev = min(len(pos_k), cap) if results is not None else 0
        if n_dev:
            out_flat[pos_k[:n_dev]] = results[k]["out"][:n_dev]
        if len(pos_k) > n_dev:
            # Overflow beyond the compiled capacity (never happens for the
            # reference input distribution) or device-fault fallback:
            # compute exactly on host.
            pos_of = pos_k[n_dev:]
            e = embed_table[ids_flat[pos_of]]
            h = _gelu_exact_np(e @ W1[k] + b1[k])
            out_flat[pos_of] = h @ W2[k] + b2[k]

    return out_flat.reshape(B, S, H)



# revision 2
# speedup vs baseline: 281.6227x; 281.6227x over previous
"""Trainium2 Bass kernel for CodecLlamaCodecEmbedding (MoE-routed per-codebook MLP).

Strategy (expert-parallel): there are 8 codebooks and 8 NeuronCores. The host
sorts tokens by codebook (the MoE dispatch) and sends core k exactly the tokens
belonging to codebook k (padded to a 128-aligned capacity so the SPMD program
is static), already gathered from the embedding table and transposed to
feature-major [16, cap] layout, plus that codebook's projector weights.

Each core runs the 2-layer projector entirely on-device:
  layer 1:  hT = gelu(W1.T @ eT + b1), f32r matmul (exact erf GELU on ScalarE
            with the bias fused), written feature-major as bf16 [2048, cap].
  layer 2:  out[tok, :] = hT.T @ W2 + b2 in bf16 x bf16 (same 1 col/cycle PE
            rate as f32r, half the HBM/SBUF traffic; measured end-to-end
            rel err ~2.8e-3 vs the 2e-2 budget). PSUM is scheduled n-major:
            one 512-column PSUM bank accumulates all 16 contraction chunks,
            drains (b2 add on VectorE, bf16 out), then the next bank starts.
            Bank reuse is 2 full n-blocks (~7 us) behind the drain, so the PE
            never waits on PSUM recycling.
W2 streams from HBM as bf16 in exactly the order tile 0 consumes it
(n-major, kc-inner), so layer 2 can start ~10 us in with no starvation.
The host upcasts the bf16 outputs and scatters them back to token order.
"""

import math
from contextlib import ExitStack

import numpy as np

import concourse.bacc as bacc
import concourse.tile as tile
from concourse import mybir
from concourse.bass_utils import run_bass_kernel_spmd

# Problem constants (hardcoded per the harness contract).
NUM_CODEBOOKS = 8
CODEBOOK_SIZE = 2048
D = 16        # codebook embedding dim
H = 2048      # hidden size
V = NUM_CODEBOOKS * CODEBOOK_SIZE  # embed table rows
N_CORES = 8

P = 128                  # SBUF partitions / tile edge
KC = H // P              # 16 contraction chunks for layer 2
NFREE = 512              # matmul moving-operand free dim (1 PSUM bank of fp32)
NSPLIT = H // NFREE      # 4 output column chunks

F32 = mybir.dt.float32
# float32r streams through the PE at full rate (1 cycle/row vs 4 for plain
# fp32 when the moving dim >= 256). Used for the tiny layer-1 matmul so the
# pre-GELU activations stay near-exact.
F32R = mybir.dt.float32r
BF16 = mybir.dt.bfloat16


def _np_bf16():
    import ml_dtypes
    return ml_dtypes.bfloat16


TUNE = {
    "group": 4,     # token tiles per layer-1 batch (N = group*128 matmuls)
    "ht_bufs": 8,
    "ob_bufs": 6,
    "l1_bufs": 2,
    "l2_bufs": 4,
}


def _emit(ctx: ExitStack, tc: tile.TileContext, aps: dict, nt: int,
          act=mybir.ActivationFunctionType.Gelu, tune=None):
    t = dict(TUNE)
    t.update(tune or {})
    group = t["group"]
    nc = tc.nc
    et_ap = aps["et"]        # [D, cap] f32r, pre-gathered transposed embeddings
    w1_ap = aps["w1"]        # [D, H]  f32r
    b1_ap = aps["b1"]        # [P, KC] f32, b1_ap[p, c] = b1[c*128 + p]
    w2_ap = aps["w2"]        # [H, H]  bf16
    b2_ap = aps["b2"]        # [P, H]  f32, b2 replicated across partitions
    out_ap = aps["out"]      # [cap, H] bf16

    const = ctx.enter_context(tc.tile_pool(name="const", bufs=1))
    w2p = ctx.enter_context(tc.tile_pool(name="w2p", bufs=1))
    htp = ctx.enter_context(tc.tile_pool(name="htp", bufs=t["ht_bufs"]))
    op = ctx.enter_context(tc.tile_pool(name="op", bufs=t["ob_bufs"]))
    l1p = ctx.enter_context(tc.tile_pool(name="l1p", bufs=t["l1_bufs"], space="PSUM"))
    l2p = ctx.enter_context(tc.tile_pool(name="l2p", bufs=t["l2_bufs"], space="PSUM"))

    # Balanced groups of <=`group` tiles, as equal as possible, so every
    # layer-1 matmul keeps a moving dim >= 256 (f32r full-rate region).
    n_groups = -(-nt // group)
    base, extra = divmod(nt, n_groups)
    sizes = [base + (1 if g < extra else 0) for g in range(n_groups)]
    starts = [sum(sizes[:g]) for g in range(n_groups)]

    # Layer-1 inputs first: w1 + the first group's embedding slice gate the
    # very first matmul, so they go ahead of everything else.
    w1_sb = const.tile([D, H], F32R)
    nc.sync.dma_start(w1_sb[:], w1_ap[:, :])
    et_sb = const.tile([D, nt * P], F32R)
    for g0, gsz in zip(starts, sizes):
        nc.sync.dma_start(et_sb[:, g0 * P:(g0 + gsz) * P],
                          et_ap[:, g0 * P:(g0 + gsz) * P])
    b1_sb = const.tile([P, KC], F32)
    nc.sync.dma_start(b1_sb[:], b1_ap[:, :])

    # W2 resident in SBUF as bf16: chunk kc occupies columns [kc*H, (kc+1)*H).
    # Stream it in the exact order tile 0's matmuls consume it (n-major,
    # kc-inner) so layer 2 never waits. b2 (1 MB) is only needed at the first
    # PSUM drain (~15 us in), so it rides between the n=0 and n=1 blocks.
    w2_sb = w2p.tile([P, KC * H], BF16)
    b2_sb = const.tile([P, H], F32)
    for n in range(NSPLIT):
        for kc in range(KC):
            nc.sync.dma_start(
                w2_sb[:, kc * H + n * NFREE:kc * H + (n + 1) * NFREE],
                w2_ap[kc * P:(kc + 1) * P, n * NFREE:(n + 1) * NFREE],
            )
        if n == 0:
            nc.sync.dma_start(b2_sb[:], b2_ap[:, :])

    def emit_l1(g):
        g0, gsz = starts[g], sizes[g]
        w = gsz * P
        eT = et_sb[:, g0 * P:g0 * P + w]
        hts = [htp.tile([P, H], BF16, tag="ht", name=f"ht_{g0 + j}")
               for j in range(gsz)]
        for hc in range(KC):
            ps1 = l1p.tile([P, group * P], F32, tag="l1")
            nc.tensor.matmul(
                ps1[:, :w],
                w1_sb[:, hc * P:(hc + 1) * P],
                eT,
                start=True,
                stop=True,
            )
            for j in range(gsz):
                nc.scalar.activation(
                    hts[j][:, hc * P:(hc + 1) * P],
                    ps1[:, j * P:(j + 1) * P],
                    act,
                    bias=b1_sb[:, hc:hc + 1],
                )
        return hts

    def emit_l2(tt, ht):
        # n-major: one PSUM bank accumulates kc=0..15, then drains while the
        # next bank accumulates. Every matmul re-issues LDWEIGHTS anyway, so
        # this order costs nothing on the PE and keeps PSUM pressure at 1.
        for n in range(NSPLIT):
            ps = l2p.tile([P, NFREE], F32, tag="l2", name=f"ps2_{tt}_{n}")
            for kc in range(KC):
                nc.tensor.matmul(
                    ps[:],
                    ht[:, kc * P:(kc + 1) * P],
                    w2_sb[:, kc * H + n * NFREE:kc * H + (n + 1) * NFREE],
                    start=(kc == 0),
                    stop=(kc == KC - 1),
                )
            ob = op.tile([P, NFREE], BF16, tag="ob")
            nc.vector.tensor_add(ob[:], ps[:], b2_sb[:, n * NFREE:(n + 1) * NFREE])
            nc.sync.dma_start(
                out_ap[tt * P:(tt + 1) * P, n * NFREE:(n + 1) * NFREE], ob[:]
            )

    # Pipeline: layer-1 for group g+1 is emitted right after the first tile of
    # group g's layer 2, so its activations overlap the remaining layer-2
    # matmuls and the hts are ready when the PE reaches them.
    hts = emit_l1(0)
    for g in range(n_groups):
        g0, gsz = starts[g], sizes[g]
        next_hts = None
        for j in range(gsz):
            emit_l2(g0 + j, hts[j])
            if j == 0 and g + 1 < n_groups:
                next_hts = emit_l1(g + 1)
        hts = next_hts


def build_nc(cap, act=mybir.ActivationFunctionType.Gelu, tune=None):
    assert cap % P == 0 and cap > 0
    nt = cap // P
    nc = bacc.Bacc("TRN2", target_bir_lowering=False, debug=False)
    aps = {
        "et": nc.dram_tensor("et", [D, cap], F32R, kind="ExternalInput").ap(),
        "w1": nc.dram_tensor("w1", [D, H], F32R, kind="ExternalInput").ap(),
        "b1": nc.dram_tensor("b1", [P, KC], F32, kind="ExternalInput").ap(),
        "w2": nc.dram_tensor("w2", [H, H], BF16, kind="ExternalInput").ap(),
        "b2": nc.dram_tensor("b2", [P, H], F32, kind="ExternalInput").ap(),
        "out": nc.dram_tensor("out", [cap, H], BF16, kind="ExternalOutput").ap(),
    }
    with tile.TileContext(nc) as tc:
        with ExitStack() as ctx:
            _emit(ctx, tc, aps, nt, act=act, tune=tune)
    nc.compile()
    return nc


_NC_CACHE = {}


def _get_nc(cap):
    if cap not in _NC_CACHE:
        _NC_CACHE[cap] = build_nc(cap)
    return _NC_CACHE[cap]


def _gelu_exact_np(x):
    try:
        from scipy.special import erf
    except ImportError:
        erf = np.vectorize(math.erf)
    return 0.5 * x * (1.0 + erf(x / np.sqrt(2.0).astype(x.dtype)))


def _route(ids_flat: np.ndarray):
    """Sort token positions by codebook. Returns per-codebook position lists."""
    cb = ids_flat // CODEBOOK_SIZE
    order = np.argsort(cb, kind="stable")
    counts = np.bincount(cb, minlength=NUM_CODEBOOKS)
    starts = np.concatenate([[0], np.cumsum(counts)])
    return [order[starts[k]:starts[k + 1]] for k in range(NUM_CODEBOOKS)], counts


MAX_DEV_CAP = 4096  # beyond this (a ~48-sigma skew) overflow tokens go to host


def pick_cap(counts):
    """Smallest multiple of 128 covering the max per-codebook load."""
    need = max(int(counts.max()), P)
    nt = -(-need // P)
    return min(nt * P, MAX_DEV_CAP)


def make_in_maps(ids_flat, embed_table, W1, b1, W2, b2, cap):
    positions, counts = _route(ids_flat)
    table = np.ascontiguousarray(embed_table, dtype=np.float32)
    bf16 = _np_bf16()
    in_maps = []
    for k in range(NUM_CODEBOOKS):
        pos_k = positions[k][:cap]
        idx_pad = np.zeros(cap, np.int64)  # padding points at table row 0
        idx_pad[:len(pos_k)] = ids_flat[pos_k]
        in_maps.append({
            "et": np.ascontiguousarray(table[idx_pad].T),
            "w1": np.ascontiguousarray(W1[k], dtype=np.float32),
            "b1": np.ascontiguousarray(np.asarray(b1[k], dtype=np.float32).reshape(KC, P).T),
            "w2": np.ascontiguousarray(np.asarray(W2[k], dtype=np.float32).astype(bf16)),
            "b2": np.ascontiguousarray(
                np.broadcast_to(np.asarray(b2[k], dtype=np.float32), (P, H))
            ),
        })
    return in_maps, positions, counts


def kernel(codec_input_ids, embed_table, W1, b1, W2, b2):
    codec_input_ids = np.asarray(codec_input_ids)
    embed_table = np.asarray(embed_table, dtype=np.float32)
    W1 = np.asarray(W1, dtype=np.float32)
    b1 = np.asarray(b1, dtype=np.float32)
    W2 = np.asarray(W2, dtype=np.float32)
    b2 = np.asarray(b2, dtype=np.float32)

    B, S = codec_input_ids.shape
    ids_flat = codec_input_ids.reshape(-1).astype(np.int64)

    _, counts = _route(ids_flat)
    cap = pick_cap(counts)
    in_maps, positions, counts = make_in_maps(
        ids_flat, embed_table, W1, b1, W2, b2, cap=cap
    )

    try:
        nc = _get_nc(cap)
        results = run_bass_kernel_spmd(nc, in_maps, list(range(N_CORES))).results
    except Exception as e:  # device/compile fault: stay correct via host math
        import sys
        print(f"kernel: device path failed ({e!r}); host fallback", file=sys.stderr)
        results = None

    out_flat = np.zeros((B * S, H), np.float32)
    for k in range(NUM_CODEBOOKS):
        pos_k = positions[k]
        n_dev = min(len(pos_k), cap) if results is not None else 0
        if n_dev:
            out_flat[pos_k[:n_dev]] = results[k]["out"][:n_dev].astype(np.float32)
        if len(pos_k) > n_dev:
            # Overflow beyond the compiled capacity (never happens for the
            # reference input distribution) or device-fault fallback:
            # compute exactly on host.
            pos_of = pos_k[n_dev:]
            e = embed_table[ids_flat[pos_of]]
            h = _gelu_exact_np(e @ W1[k] + b1[k])
            out_flat[pos_of] = h @ W2[k] + b2[k]

    return out_flat.reshape(B, S, H)
